# revision 10
# baseline (speedup 1.0000x reference)
"""Trainium2 Bass kernel for nn_BlockBERTlucidrains (block-recurrent BERT fwd).

Returns (log_softmax_out, state_out) matching the reference:
  - log_softmax over a size-1 axis is exactly zeros -> computed on host.
  - state_out depends only on layers 0,1 and the k/v + state path of layer 2
    (layer 3, final LN, head, and layer-2 token-attention are dead code).

Sharding: data-parallel over batch, 4 sequences per core x 8 cores,
weights replicated, no collectives.
"""
import numpy as np
import ml_dtypes

# problem dims (hardcoded per spec)
V, B, N, D, L, H = 30522, 32, 512, 512, 4, 8
HD = D // H
REC = L // 2
SCALE = float(HD) ** -0.5
NCORES = 8
BLOC = B // NCORES      # 4 sequences per core
TOK = BLOC * N          # 2048 token rows per core
P = 128
NT = TOK // P           # 16 token tiles per core
TPS = N // P            # 4 tiles per sequence
DC = D // P             # 4 chunks of d_model
FC = (4 * D) // P       # 16 chunks of ffn dim
SQC = (3 * D) // P      # 12 chunks of state-qkv rows

BF16 = ml_dtypes.bfloat16

_CACHE = {}


def _build(flags):
    """Build + compile the SPMD program. flags = (has_ln1b, has_bff2, has_slnb)."""
    import concourse.bacc as bacc
    import concourse.bass as bass
    import concourse.tile as tile
    import concourse.mybir as mybir

    has_ln1b, has_bff2, has_slnb = flags
    F32 = mybir.dt.float32
    BF = mybir.dt.bfloat16
    I32 = mybir.dt.int32
    Alu = mybir.AluOpType
    Act = mybir.ActivationFunctionType

    nc = bacc.Bacc()

    # ---- DRAM parameters (per core) ----
    dp = lambda name, shape, dt: nc.declare_dram_parameter(name, list(shape), dt,
                                                           isOutput=False)
    ids_d = dp("ids_t", (P, NT), I32)
    state_d = dp("state", (BLOC, D), F32)
    tok_d = dp("tok_emb", (V, D), F32)
    cos_d = dp("cos", (P, TPS, HD // 2), F32)
    sin_d = dp("sinsgn", (P, TPS, 2, HD // 2), F32)
    msk_d = dp("negtriu", (P, P), F32)
    idn_d = dp("ident", (P, P), BF)
    gate_d = dp("gate", (BLOC, D), F32)
    wqkv_d = [dp(f"wqkv{l}", (P, DC, 3 * D), BF) for l in range(2)]
    wo_d = [dp(f"wo{l}", (P, DC, D), BF) for l in range(2)]
    wff1_d = [dp(f"wff1{l}", (P, DC, 4 * D), BF) for l in range(2)]
    bff1_d = [dp(f"bff1{l}", (P, FC), F32) for l in range(2)]
    wff2_d = [dp(f"wff2{l}", (P, FC, D), BF) for l in range(2)]
    wkv2_d = dp("wkv2", (P, DC, 2 * D), BF)
    wsq_d = dp("wsq", (P, DC, 3 * D), BF)
    wso_d = dp("wso", (P, DC, D), BF)
    if has_ln1b:
        qb_d = [dp(f"qkvbias{l}", (1, 3 * D), BF) for l in range(2)]
        kvb_d = dp("kvbias2", (1, 2 * D), BF)
    if has_bff2:
        b2_d = [dp(f"bff2{l}", (1, D), BF) for l in range(2)]
    if has_slnb:
        sb_d = dp("sbias", (P, SQC), F32)
        svb_d = dp("svbias", (1, D), BF)
    out_d = nc.declare_dram_parameter("state_out", [BLOC, D], F32, isOutput=True)

    with tile.TileContext(nc) as tc:
        with (
            tc.tile_pool(name="const", bufs=1) as cp,
            tc.tile_pool(name="xp", bufs=1) as xp,
            tc.tile_pool(name="wq", bufs=2) as wq_pool,
            tc.tile_pool(name="wo", bufs=1) as wo_pool,
            tc.tile_pool(name="wf1", bufs=1) as wf1_pool,
            tc.tile_pool(name="wf2", bufs=1) as wf2_pool,
            tc.tile_pool(name="work", bufs=2) as wk,
            tc.tile_pool(name="seq", bufs=2) as sq,
            tc.tile_pool(name="ftp", bufs=1) as ftp,
            tc.tile_pool(name="ps_big", bufs=3, space="PSUM") as psb,
            tc.tile_pool(name="ps_tr", bufs=2, space="PSUM") as pst_pool,
            tc.tile_pool(name="ps_sm", bufs=3, space="PSUM") as pss_pool,
        ):
            # ---- constants ----
            cos_sb = cp.tile([P, TPS, HD // 2], F32)
            nc.sync.dma_start(cos_sb[:], cos_d[:])
            sin_sb = cp.tile([P, TPS, 2, HD // 2], F32)
            nc.sync.dma_start(sin_sb[:], sin_d[:])
            msk_sb = cp.tile([P, P], F32)
            nc.sync.dma_start(msk_sb[:], msk_d[:])
            idn_sb = cp.tile([P, P], BF)
            nc.sync.dma_start(idn_sb[:], idn_d[:])
            gate_sb = cp.tile([BLOC, D], F32)
            nc.sync.dma_start(gate_sb[:], gate_d[:])
            eps_sb = cp.tile([P, 1], F32)
            nc.vector.memset(eps_sb[:], 1e-5)
            ids_sb = cp.tile([P, NT], I32)
            nc.sync.dma_start(ids_sb[:], ids_d[:])
            st_sb = cp.tile([BLOC, D], F32)
            nc.sync.dma_start(st_sb[:], state_d[:])
            bff1_sb = [cp.tile([P, FC], F32, name=f"bff1sb{l}") for l in range(2)]
            for l in range(2):
                nc.sync.dma_start(bff1_sb[l][:], bff1_d[l][:])
            if has_ln1b:
                ones1_sb = cp.tile([1, P], BF)
                nc.vector.memset(ones1_sb[:], 1.0)
                qb_sb = [cp.tile([1, 3 * D], BF, name=f"qbsb{l}") for l in range(2)]
                for l in range(2):
                    nc.sync.dma_start(qb_sb[l][:], qb_d[l][:])
                kvb_sb = cp.tile([1, 2 * D], BF)
                nc.sync.dma_start(kvb_sb[:], kvb_d[:])
            if has_bff2:
                if not has_ln1b:
                    ones1_sb = cp.tile([1, P], BF)
                    nc.vector.memset(ones1_sb[:], 1.0)
                b2_sb = [cp.tile([1, D], BF, name=f"b2sb{l}") for l in range(2)]
                for l in range(2):
                    nc.sync.dma_start(b2_sb[l][:], b2_d[l][:])
            if has_slnb:
                if not (has_ln1b or has_bff2):
                    ones1_sb = cp.tile([1, P], BF)
                    nc.vector.memset(ones1_sb[:], 1.0)
                sb_sb = cp.tile([P, SQC], F32)
                nc.sync.dma_start(sb_sb[:], sb_d[:])
                svb_sb = cp.tile([1, D], BF)
                nc.sync.dma_start(svb_sb[:], svb_d[:])

            # ---- persistent x activations [P, NT, D] f32 ----
            x_sb = xp.tile([P, NT, D], F32)
            for t in range(NT):
                nc.gpsimd.indirect_dma_start(
                    out=x_sb[:, t, :], out_offset=None, in_=tok_d[:],
                    in_offset=bass.IndirectOffsetOnAxis(ap=ids_sb[:, t:t + 1],
                                                        axis=0))

            # ---- per-layer weight tiles ----
            def load_w(l):
                w = {}
                w["qkv"] = wq_pool.tile([P, DC, 3 * D], BF, tag="wqkv",
                                        name=f"wqkv{l}")
                nc.sync.dma_start(w["qkv"][:], wqkv_d[l][:])
                w["wo"] = wo_pool.tile([P, DC, D], BF, tag="wo",
                                       name=f"wo{l}")
                nc.sync.dma_start(w["wo"][:], wo_d[l][:])
                w["ff1"] = wf1_pool.tile([P, DC, 4 * D], BF, tag="wff1",
                                         name=f"wff1{l}")
                nc.sync.dma_start(w["ff1"][:], wff1_d[l][:])
                w["ff2"] = wf2_pool.tile([P, FC, D], BF, tag="wff2",
                                         name=f"wff2{l}")
                nc.sync.dma_start(w["ff2"][:], wff2_d[l][:])
                return w

            def layernorm(src_ap, name):
                """LN (g/b folded elsewhere): returns bf16 [P, D] tile."""
                stats = wk.tile([P, 6], F32, tag="lnstats", name=f"st{name}")
                nc.vector.bn_stats(stats[:], src_ap)
                mv = wk.tile([P, 2], F32, tag="lnmv", name=f"mv{name}")
                nc.vector.bn_aggr(mv[:], stats[:])
                r = wk.tile([P, 1], F32, tag="lnr", name=f"r{name}")
                nc.scalar.activation(r[:], mv[:, 1:2], Act.Sqrt,
                                     bias=eps_sb[:, :1], scale=1.0)
                nc.vector.reciprocal(r[:], r[:])
                h = wk.tile([P, D], BF, tag="h", name=f"h{name}")
                nc.vector.tensor_scalar(out=h[:], in0=src_ap,
                                        scalar1=mv[:, 0:1], scalar2=r[:],
                                        op0=Alu.subtract, op1=Alu.mult)
                return h

            def transpose_to(dst_ap_fn, src, nchunks, pdim=P):
                """PE-transpose src [pdim, nchunks*P] bf16 into dst chunks."""
                for c in range(nchunks):
                    pt = pst_pool.tile([P, pdim], BF, tag="tr", name=f"tr{c}")
                    nc.tensor.transpose(pt[:], src[:, c * P:(c + 1) * P],
                                        idn_sb[:pdim, :pdim])
                    nc.vector.tensor_copy(dst_ap_fn(c), pt[:])

            def rope(ps, tt, name):
                """ps: [P, D] psum -> bf16 [P, D] tile (rotary, token-major)."""
                J = HD // 2
                t1 = wk.tile([P, D], F32, tag="ropet1", name=f"a{name}")
                p4 = ps[:].rearrange("p (h two j) -> p h two j", two=2, j=J)
                t14 = t1[:].rearrange("p (h two j) -> p h two j", two=2, j=J)
                sin_t1 = sin_sb[:, tt, 0:1, :].to_broadcast([P, H, J])
                sin_t2 = sin_sb[:, tt, 1:2, :].to_broadcast([P, H, J])
                nc.vector.tensor_tensor(out=t14[:, :, 0, :], in0=p4[:, :, 1, :],
                                        in1=sin_t1, op=Alu.mult)
                nc.vector.tensor_tensor(out=t14[:, :, 1, :], in0=p4[:, :, 0, :],
                                        in1=sin_t2, op=Alu.mult)
                t2 = wk.tile([P, D], F32, tag="ropet2", name=f"b{name}")
                cos_b = cos_sb[:, tt, None, None, :].to_broadcast([P, H, 2, J])
                nc.vector.tensor_tensor(
                    out=t2[:].rearrange("p (h two j) -> p h two j", two=2, j=J),
                    in0=p4[:], in1=cos_b, op=Alu.mult)
                o = wk.tile([P, D], BF, tag="ropeo", name=f"c{name}")
                nc.vector.tensor_tensor(out=o[:], in0=t1[:], in1=t2[:],
                                        op=Alu.add)
                return o

            def qkv_bias_seed(psum, bias_row_ap):
                nc.tensor.matmul(psum[:], lhsT=ones1_sb[:, :], rhs=bias_row_ap,
                                 start=True, stop=False)

            def attention(s, qT, kT, vaug, attn):
                for h in range(H):
                    hc, po = h // 2, (h % 2) * HD
                    PT = sq.tile([P, TPS, N], BF, tag="PT", name=f"PT{s}_{h}")
                    for kb in range(TPS):
                        valid = N - kb * P
                        pssc = psb.tile([P, 512], F32, tag="big",
                                        name=f"sc{s}_{h}_{kb}")
                        nc.tensor.matmul(
                            pssc[:, :valid],
                            lhsT=kT[po:po + HD, hc, kb * P:(kb + 1) * P],
                            rhs=qT[po:po + HD, hc, kb * P:N],
                            start=True, stop=True)
                        nc.vector.tensor_tensor(out=pssc[:, :P], in0=pssc[:, :P],
                                                in1=msk_sb[:], op=Alu.add)
                        nc.scalar.activation(PT[:, kb, kb * P:N], pssc[:, :valid],
                                             Act.Exp, scale=SCALE)
                    for qt in range(TPS):
                        pso = pss_pool.tile([P, HD + 1], F32, tag="sm",
                                            name=f"av{s}_{h}_{qt}")
                        for kb in range(qt + 1):
                            nc.tensor.matmul(pso[:],
                                             lhsT=PT[:, kb, qt * P:(qt + 1) * P],
                                             rhs=vaug[:, kb, h, :],
                                             start=(kb == 0), stop=(kb == qt))
                        r = wk.tile([P, 1], F32, tag="avr", name=f"avr{s}{h}{qt}")
                        nc.vector.reciprocal(r[:], pso[:, HD:HD + 1])
                        nc.vector.tensor_scalar_mul(
                            out=attn[:, qt, h * HD:(h + 1) * HD],
                            in0=pso[:, :HD], scalar1=r[:])

            # =================== layers 0, 1 (full) ===================
            for l in range(2):
                w = load_w(l)
                qT = {}; kT = {}; vaug = {}; attn = {}; h2T = {}
                for s in range(BLOC):
                    qT[s] = sq.tile([P, DC, N], BF, tag="qT", name=f"qT{l}{s}")
                    kT[s] = sq.tile([P, DC, N], BF, tag="kT", name=f"kT{l}{s}")
                    vaug[s] = sq.tile([P, TPS, H, HD + 1], BF, tag="vaug",
                                      name=f"va{l}{s}")
                    nc.gpsimd.memset(vaug[s][:, :, :, HD:HD + 1], 1.0)
                    for tt in range(TPS):
                        t = s * TPS + tt
                        h1 = layernorm(x_sb[:, t, :], f"l{l}t{t}")
                        hT = wk.tile([P, DC, P], BF, tag="hT", name=f"hT{l}{t}")
                        transpose_to(lambda c: hT[:, c, :], h1, DC)
                        pq = psb.tile([P, 512], F32, tag="big", name=f"pq{l}{t}")
                        pk = psb.tile([P, 512], F32, tag="big", name=f"pk{l}{t}")
                        pv = psb.tile([P, 512], F32, tag="big", name=f"pv{l}{t}")
                        for (pp, lo) in ((pq, 0), (pk, D), (pv, 2 * D)):
                            if has_ln1b:
                                qkv_bias_seed(pp, qb_sb[l][:, lo:lo + D])
                            for c in range(DC):
                                nc.tensor.matmul(
                                    pp[:], lhsT=hT[:, c, :],
                                    rhs=w["qkv"][:, c, lo:lo + D],
                                    start=(c == 0 and not has_ln1b),
                                    stop=(c == DC - 1))
                        qbf = rope(pq, tt, f"q{l}{t}")
                        kbf = rope(pk, tt, f"k{l}{t}")
                        nc.vector.tensor_copy(
                            vaug[s][:, tt, :, 0:HD],
                            pv[:].rearrange("p (h d) -> p h d", h=H))
                        transpose_to(lambda c: qT[s][:, c, tt * P:(tt + 1) * P],
                                     qbf, DC)
                        transpose_to(lambda c: kT[s][:, c, tt * P:(tt + 1) * P],
                                     kbf, DC)
                    attn[s] = sq.tile([P, TPS, D], BF, tag="attn",
                                      name=f"at{l}{s}")
                    attention(s, qT[s], kT[s], vaug[s], attn[s])
                    # wo + residual + LN2 + h2T
                    h2T[s] = sq.tile([P, DC, N], BF, tag="h2T", name=f"h2T{l}{s}")
                    for tt in range(TPS):
                        t = s * TPS + tt
                        aT = wk.tile([P, DC, P], BF, tag="hT", name=f"aT{l}{t}")
                        transpose_to(lambda c: aT[:, c, :], attn[s][:, tt, :], DC)
                        px = psb.tile([P, 512], F32, tag="big", name=f"px{l}{t}")
                        for c in range(DC):
                            nc.tensor.matmul(px[:], lhsT=aT[:, c, :],
                                             rhs=w["wo"][:, c, :],
                                             start=(c == 0), stop=(c == DC - 1))
                        nc.vector.tensor_tensor(out=x_sb[:, t, :],
                                                in0=x_sb[:, t, :], in1=px[:],
                                                op=Alu.add)
                        h2 = layernorm(x_sb[:, t, :], f"n2{l}t{t}")
                        transpose_to(
                            lambda c: h2T[s][:, c, tt * P:(tt + 1) * P], h2, DC)
                    # ffn
                    fT = ftp.tile([P, FC, N], BF, tag="fT", name=f"fT{l}{s}")
                    for fc in range(FC):
                        pf = psb.tile([P, 512], F32, tag="big",
                                      name=f"pf{l}{s}{fc}")
                        for c in range(DC):
                            nc.tensor.matmul(
                                pf[:], lhsT=w["ff1"][:, c, fc * P:(fc + 1) * P],
                                rhs=h2T[s][:, c, :],
                                start=(c == 0), stop=(c == DC - 1))
                        nc.scalar.activation(fT[:, fc, :], pf[:], Act.Gelu,
                                             bias=bff1_sb[l][:, fc:fc + 1],
                                             scale=1.0)
                    for tt in range(TPS):
                        t = s * TPS + tt
                        p2 = psb.tile([P, 512], F32, tag="big", name=f"p2{l}{t}")
                        if has_bff2:
                            qkv_bias_seed(p2, b2_sb[l][:, :])
                        for fc in range(FC):
                            nc.tensor.matmul(
                                p2[:], lhsT=fT[:, fc, tt * P:(tt + 1) * P],
                                rhs=w["ff2"][:, fc, :],
                                start=(fc == 0 and not has_bff2),
                                stop=(fc == FC - 1))
                        nc.vector.tensor_tensor(out=x_sb[:, t, :],
                                                in0=x_sb[:, t, :], in1=p2[:],
                                                op=Alu.add)

            # =================== layer 2: state path prep ===================
            wkv2_sb = wq_pool.tile([P, DC, 3 * D], BF, tag="wqkv")
            nc.sync.dma_start(wkv2_sb[:, :, :2 * D], wkv2_d[:])
            wsq_sb = wf1_pool.tile([P, DC, 4 * D], BF, tag="wff1")
            nc.sync.dma_start(wsq_sb[:, :, :3 * D], wsq_d[:])
            wso_sb = wo_pool.tile([P, DC, D], BF, tag="wo")
            nc.sync.dma_start(wso_sb[:], wso_d[:])

            # state LN + sqkv (transposed) + sv (token-major)
            snstats = cp.tile([BLOC, 6], F32)
            nc.vector.bn_stats(snstats[:], st_sb[:])
            snmv = cp.tile([BLOC, 2], F32)
            nc.vector.bn_aggr(snmv[:], snstats[:])
            snr = cp.tile([BLOC, 1], F32)
            nc.scalar.activation(snr[:], snmv[:, 1:2], Act.Sqrt,
                                 bias=eps_sb[:BLOC, :1], scale=1.0)
            nc.vector.reciprocal(snr[:], snr[:])
            snbf = cp.tile([BLOC, D], BF)
            nc.vector.tensor_scalar(out=snbf[:], in0=st_sb[:],
                                    scalar1=snmv[:, 0:1], scalar2=snr[:],
                                    op0=Alu.subtract, op1=Alu.mult)
            snT = cp.tile([P, DC, BLOC], BF)
            for c in range(DC):
                ptn = pst_pool.tile([P, BLOC], BF, tag="tr", name=f"sn{c}")
                nc.tensor.transpose(ptn[:], snbf[:, c * P:(c + 1) * P],
                                    idn_sb[:BLOC, :BLOC])
                nc.vector.tensor_copy(snT[:, c, :], ptn[:])
            sqkvT = cp.tile([P, SQC, BLOC], BF)
            for fc in range(SQC):
                pq = pss_pool.tile([P, BLOC], F32, tag="sm", name=f"sq{fc}")
                for c in range(DC):
                    nc.tensor.matmul(pq[:],
                                     lhsT=wsq_sb[:, c, fc * P:(fc + 1) * P],
                                     rhs=snT[:, c, :],
                                     start=(c == 0), stop=(c == DC - 1))
                if has_slnb:
                    nc.vector.tensor_scalar(out=sqkvT[:, fc, :], in0=pq[:],
                                            scalar1=sb_sb[:, fc:fc + 1],
                                            scalar2=None, op0=Alu.add)
                else:
                    nc.vector.tensor_copy(sqkvT[:, fc, :], pq[:])
            # sv token-major [BLOC, D] (+ones cols) -> per-seq rows at part 0
            psv4 = pss_pool.tile([BLOC, 512], F32, tag="sm", name="psv4")
            if has_slnb:
                nc.tensor.matmul(psv4[:], lhsT=ones1_sb[:, :BLOC],
                                 rhs=svb_sb[:], start=True, stop=False)
            for c in range(DC):
                nc.tensor.matmul(psv4[:], lhsT=snT[:, c, :],
                                 rhs=wsq_sb[:, c, 2 * D:3 * D],
                                 start=(c == 0 and not has_slnb),
                                 stop=(c == DC - 1))
            svtm = cp.tile([BLOC, H, HD + 1], BF)
            nc.vector.tensor_copy(svtm[:, :, :HD],
                                  psv4[:].rearrange("p (h d) -> p h d", h=H))
            nc.vector.memset(svtm[:, :, HD:HD + 1], 1.0)
            svrows = cp.tile([1, BLOC, H, HD + 1], BF)
            for s in range(BLOC):
                nc.sync.dma_start(svrows[0:1, s], svtm[s:s + 1])

            updrows = cp.tile([1, BLOC, D], BF)

            # ---- layer 2 token k/v + per-seq state attention ----
            for s in range(BLOC):
                kT2 = sq.tile([P, DC, N], BF, tag="kT", name=f"kT2{s}")
                va2 = sq.tile([P, TPS, H, HD + 1], BF, tag="vaug",
                              name=f"va2{s}")
                nc.gpsimd.memset(va2[:, :, :, HD:HD + 1], 1.0)
                for tt in range(TPS):
                    t = s * TPS + tt
                    h1 = layernorm(x_sb[:, t, :], f"l2t{t}")
                    hT = wk.tile([P, DC, P], BF, tag="hT", name=f"hT2{t}")
                    transpose_to(lambda c: hT[:, c, :], h1, DC)
                    pk = psb.tile([P, 512], F32, tag="big", name=f"pk2{t}")
                    pv = psb.tile([P, 512], F32, tag="big", name=f"pv2{t}")
                    for (pp, lo) in ((pk, 0), (pv, D)):
                        if has_ln1b:
                            qkv_bias_seed(pp, kvb_sb[:, lo:lo + D])
                        for c in range(DC):
                            nc.tensor.matmul(pp[:], lhsT=hT[:, c, :],
                                             rhs=wkv2_sb[:, c, lo:lo + D],
                                             start=(c == 0 and not has_ln1b),
                                             stop=(c == DC - 1))
                    kbf = rope(pk, tt, f"k2{t}")
                    nc.vector.tensor_copy(
                        va2[:, tt, :, 0:HD],
                        pv[:].rearrange("p (h d) -> p h d", h=H))
                    transpose_to(lambda c: kT2[:, c, tt * P:(tt + 1) * P],
                                 kbf, DC)
                # state attention for sequence s
                for h in range(H):
                    hc, po = h // 2, (h % 2) * HD
                    pexp = wk.tile([P, TPS], BF, tag="pexp", name=f"pe{s}{h}")
                    for kb in range(TPS):
                        pst1 = pss_pool.tile([P, 1], F32, tag="sm",
                                             name=f"ss{s}{h}{kb}")
                        nc.tensor.matmul(
                            pst1[:],
                            lhsT=kT2[po:po + HD, hc, kb * P:(kb + 1) * P],
                            rhs=sqkvT[po:po + HD, hc, s:s + 1],
                            start=True, stop=True)
                        nc.scalar.activation(pexp[:, kb:kb + 1], pst1[:],
                                             Act.Exp, scale=SCALE)
                    psk1 = pss_pool.tile([1, 1], F32, tag="sm",
                                         name=f"sk{s}{h}")
                    nc.tensor.matmul(psk1[:],
                                     lhsT=sqkvT[po:po + HD, 4 + hc, s:s + 1],
                                     rhs=sqkvT[po:po + HD, hc, s:s + 1],
                                     start=True, stop=True)
                    skexp = wk.tile([1, 1], BF, tag="skexp", name=f"se{s}{h}")
                    nc.scalar.activation(skexp[:], psk1[:], Act.Exp,
                                         scale=SCALE)
                    psu = pss_pool.tile([1, HD + 1], F32, tag="sm",
                                        name=f"su{s}{h}")
                    nc.tensor.matmul(psu[:], lhsT=skexp[:],
                                     rhs=svrows[0:1, s, h, :],
                                     start=True, stop=False)
                    for kb in range(TPS):
                        nc.tensor.matmul(psu[:], lhsT=pexp[:, kb:kb + 1],
                                         rhs=va2[:, kb, h, :],
                                         start=False, stop=(kb == TPS - 1))
                    ru = wk.tile([1, 1], F32, tag="ru", name=f"ru{s}{h}")
                    nc.vector.reciprocal(ru[:], psu[0:1, HD:HD + 1])
                    nc.vector.tensor_scalar_mul(
                        out=updrows[0:1, s, h * HD:(h + 1) * HD],
                        in0=psu[0:1, :HD], scalar1=ru[:])

            # ---- upd @ w_so, gate, output ----
            updT = cp.tile([P, DC, BLOC], BF)
            for s in range(BLOC):
                for c in range(DC):
                    nc.sync.dma_start(
                        updT[:, c, s].opt(),
                        updrows[0:1, s, c * P:(c + 1) * P].opt())
            pupd = pss_pool.tile([BLOC, D], F32, tag="sm", name="pupd")
            for c in range(DC):
                nc.tensor.matmul(pupd[:], lhsT=updT[:, c, :],
                                 rhs=wso_sb[:, c, :],
                                 start=(c == 0), stop=(c == DC - 1))
            dd = cp.tile([BLOC, D], F32)
            nc.vector.tensor_tensor(out=dd[:], in0=st_sb[:], in1=pupd[:],
                                    op=Alu.subtract)
            nc.vector.tensor_tensor(out=dd[:], in0=dd[:], in1=gate_sb[:],
                                    op=Alu.mult)
            so = cp.tile([BLOC, D], F32)
            nc.vector.tensor_tensor(out=so[:], in0=dd[:], in1=pupd[:],
                                    op=Alu.add)
            nc.sync.dma_start(out_d[:], so[:])

    nc.compile()
    return nc


def _prep(inputs):
    f32 = np.float32
    g = {k: np.asarray(v) for k, v in inputs.items()}
    ids = g["ids"].astype(np.int32)
    state = g["state"].astype(f32)
    tok = np.ascontiguousarray(g["tok_emb"], dtype=f32)
    wqkv = g["w_qkv"].astype(f32)
    ln1g, ln1b = g["ln1_g"].astype(f32), g["ln1_b"].astype(f32)
    ln2g, ln2b = g["ln2_g"].astype(f32), g["ln2_b"].astype(f32)
    wo = g["w_o"].astype(f32)
    wff1, bff1 = g["w_ff1"].astype(f32), g["b_ff1"].astype(f32)
    wff2, bff2 = g["w_ff2"].astype(f32), g["b_ff2"].astype(f32)
    slng, slnb = g["s_ln_g"].astype(f32), g["s_ln_b"].astype(f32)
    wsq = g["w_s_qkv"].astype(f32)

    def chunked(w, dt=BF16):  # [D, F] -> [P, DC, F]
        d0, F = w.shape
        c = d0 // P
        return np.ascontiguousarray(w.reshape(c, P, F).transpose(1, 0, 2)
                                    ).astype(dt)

    sh = {}
    for l in range(2):
        sh[f"wqkv{l}"] = chunked(ln1g[l][:, None] * wqkv[l])
        sh[f"wo{l}"] = chunked(wo[l])
        sh[f"wff1{l}"] = chunked(ln2g[l][:, None] * wff1[l])
        btot = ln2b[l] @ wff1[l] + bff1[l]
        sh[f"bff1{l}"] = np.ascontiguousarray(btot.reshape(FC, P).T
                                              ).astype(f32)
        sh[f"wff2{l}"] = np.ascontiguousarray(
            wff2[l].reshape(FC, P, D).transpose(1, 0, 2)).astype(BF16)
    sh["wkv2"] = chunked(ln1g[2][:, None] * wqkv[2][:, D:])
    sh["wsq"] = chunked(slng[:, None] * wsq)
    sh["wso"] = chunked(g["w_so"].astype(f32))
    gate = 1.0 / (1.0 + np.exp(-g["gate_beta"].astype(f32)))
    sh["gate"] = np.ascontiguousarray(np.broadcast_to(gate, (BLOC, D)),
                                      dtype=f32)

    inv = 1.0 / (10000.0 ** (np.arange(0, HD, 2, dtype=f32) / HD))
    ang = np.arange(N, dtype=f32)[:, None] * inv            # [N, 32]
    cosm, sinm = np.cos(ang).astype(f32), np.sin(ang).astype(f32)
    J = HD // 2
    sh["cos"] = np.ascontiguousarray(
        cosm.reshape(TPS, P, J).transpose(1, 0, 2)).astype(f32)
    sinsgn = np.stack([-sinm, sinm], axis=1)                # [N, 2, 32]
    sh["sinsgn"] = np.ascontiguousarray(
        sinsgn.reshape(TPS, P, 2, J).transpose(1, 0, 2, 3)).astype(f32)
    ii, jj = np.meshgrid(np.arange(P), np.arange(P), indexing="ij")
    sh["negtriu"] = np.where(jj < ii, -1e9, 0.0).astype(f32)
    sh["ident"] = np.eye(P, dtype=BF16)
    sh["tok_emb"] = tok

    has_ln1b = bool(np.any(ln1b[:3] != 0))
    has_bff2 = bool(np.any(bff2[:2] != 0))
    has_slnb = bool(np.any(slnb != 0))
    if has_ln1b:
        for l in range(2):
            sh[f"qkvbias{l}"] = (ln1b[l] @ wqkv[l]).reshape(1, 3 * D
                                                           ).astype(BF16)
        sh["kvbias2"] = (ln1b[2] @ wqkv[2][:, D:]).reshape(1, 2 * D
                                                          ).astype(BF16)
    if has_bff2:
        for l in range(2):
            sh[f"bff2{l}"] = bff2[l].reshape(1, D).astype(BF16)
    if has_slnb:
        sbias = slnb @ wsq
        sh["sbias"] = np.ascontiguousarray(sbias.reshape(SQC, P).T
                                           ).astype(f32)
        sh["svbias"] = sbias[2 * D:].reshape(1, D).astype(BF16)

    in_maps = []
    for core in range(NCORES):
        m = dict(sh)
        idc = ids[core * BLOC:(core + 1) * BLOC].reshape(TOK)
        m["ids_t"] = np.ascontiguousarray(idc.reshape(NT, P).T)
        m["state"] = np.ascontiguousarray(state[core * BLOC:(core + 1) * BLOC])
        in_maps.append(m)
    return in_maps, (has_ln1b, has_bff2, has_slnb)


def _run(inputs, trace=False):
    from concourse.bass_utils import run_bass_kernel_spmd
    in_maps, flags = _prep(inputs)
    if flags not in _CACHE:
        _CACHE[flags] = _build(flags)
    nc = _CACHE[flags]
    res = run_bass_kernel_spmd(nc, in_maps, list(range(NCORES)), trace=trace,
                               trace_cores=[0] if trace else None)
    state_out = np.concatenate([res.results[i]["state_out"]
                                for i in range(NCORES)], axis=0)
    out0 = np.zeros((B, N, 1), np.float32)
    return (out0, state_out), res


def kernel(**inputs):
    (out0, state_out), _ = _run(inputs, trace=False)
    return out0, state_out
